# revision 3
# baseline (speedup 1.0000x reference)
"""Trainium2 Bass kernel for the Event-SMLP-Quantized SNN — v2.

Data-parallel over batch: 1024 per core x 8 cores. Changes vs v1:
  - rand_u transposed to feature-major on the HOST -> no on-device
    transposes; the is_gt compare directly yields xT [k, b] tiles.
  - Layer-2 runs a single fp32r pass (PE truncates f32->FP22; W2 is
    pre-truncated so products are exact w.r.t. the truncated weights).
  - Layer-1 hi split uses fp22-truncated W1 via fp32r (exact products),
    lo split = bf16 residual -> product error ~2^-21 (vs 2^-17 for the
    plain 2-way bf16 split).
  - Software-pipelined emission: next-t compares and DMAs are slotted
    mid-timestep to keep the PE stream gapless.

Per timestep, per core:
  x = (sample > rand_t)            DVE compare, feature-major, bf16 out
  u = 0.2*m1 + x @ W1.T            ACT preload + 14 PE matmuls / h-tile
  s1_inv = (u <= 0.5); m1 = u*s1_inv
  z2 = rowsum(W2eff) + 0.2*m2 + s1_inv @ (-W2eff).T   (8 fp32r matmuls)
  s2_inv = (z2 <= 0.5); m2 = z2*s2_inv; ssinv += s2_inv
  out = 1 - ssinv/T
"""
import os
import sys

sys.path.insert(0, "/opt/trn_rl_repo")

import numpy as np
import ml_dtypes

import concourse.bacc as bacc
import concourse.mybir as mybir
from concourse import tile
from concourse.bass_utils import run_bass_kernel_spmd

F32 = mybir.dt.float32
F32R = mybir.dt.float32r
BF16 = mybir.dt.bfloat16
AOP = mybir.AluOpType
AFT = mybir.ActivationFunctionType

NCORES = 8
B_FULL = 8192
BL = B_FULL // NCORES        # 1024 batch per core
NBT = 2                      # matmul N-tiles of 512
BW = 512
K = 784
KPAD = 896                   # 7 * 128
KC = KPAD // 128             # 7 contraction chunks
H = 1024
NH = H // 128                # 8 hidden tiles
O = 10
DECAY = 0.2
THRESH = 0.5

# L1 hi-split carried as fp32r (fp22-exact) when trunc model confirmed,
# else plain bf16. Set by experiment results.
HI_MODE = os.environ.get("KERNEL_HI_MODE", "bf16")   # "fp22" | "bf16"

_cache = {}


def _build(T):
    nc = bacc.Bacc("TRN2", target_bir_lowering=False, debug=False)

    d_rand = nc.declare_dram_parameter("rand", [T, NBT, KPAD, BW], F32, isOutput=False)
    d_sample = nc.declare_dram_parameter("sampleT", [128, KC, BL], F32, isOutput=False)
    hi_dt = F32R if HI_MODE == "fp22" else BF16
    d_w1hi = nc.declare_dram_parameter("w1hi", [128, KC, H], hi_dt, isOutput=False)
    d_w1lo = nc.declare_dram_parameter("w1lo", [128, KC, H], BF16, isOutput=False)
    d_w2 = nc.declare_dram_parameter("w2", [128, NH, O], F32R, isOutput=False)
    d_rs2 = nc.declare_dram_parameter("rs2", [O, 1], F32, isOutput=False)
    d_out = nc.declare_dram_parameter("out", [O, BL], F32, isOutput=True)

    with tile.TileContext(nc) as tc:
        with (
            tc.tile_pool(name="const", bufs=1) as cpool,
            tc.tile_pool(name="state", bufs=1) as spool,
            tc.tile_pool(name="rand", bufs=3) as rpool,
            tc.tile_pool(name="x", bufs=3) as xpool,
            tc.tile_pool(name="s1", bufs=10) as s1pool,
            tc.tile_pool(name="s2", bufs=4) as s2pool,
            tc.tile_pool(name="ps1", bufs=6, space="PSUM") as ps1pool,
            tc.tile_pool(name="ps2", bufs=2, space="PSUM") as ps2pool,
        ):
            sampleT = cpool.tile([128, KC, BL], F32, tag="sampleT")
            nc.sync.dma_start(out=sampleT[:], in_=d_sample[:])
            w1hi = cpool.tile([128, KC, H], hi_dt, tag="w1hi")
            nc.sync.dma_start(out=w1hi[:], in_=d_w1hi[:])
            w1lo = cpool.tile([128, KC, H], BF16, tag="w1lo")
            nc.sync.dma_start(out=w1lo[:], in_=d_w1lo[:])
            w2 = cpool.tile([128, NH, O], F32R, tag="w2")
            nc.sync.dma_start(out=w2[:], in_=d_w2[:])
            rs2 = cpool.tile([O, 1], F32, tag="rs2")
            nc.sync.dma_start(out=rs2[:], in_=d_rs2[:])

            dstate = []
            for h in range(NH):
                d_ = spool.tile([128, BL], F32, tag=f"dm{h}")
                nc.vector.memset(d_[:], 0.0)
                dstate.append(d_)
            d2 = spool.tile([O, BL], F32, tag="dl2")
            nc.vector.memset(d2[:], 0.0)
            ssinv = spool.tile([O, BL], F32, tag="ssinv")
            nc.vector.memset(ssinv[:], 0.0)

            def emit_dma(t, bt):
                rt = rpool.tile([128, KC, BW], F32, tag="rand")
                src = d_rand[t, bt].rearrange("(c p) b -> p c b", p=128)
                nc.sync.dma_start(out=rt[:], in_=src)
                return rt

            def emit_compare(rt, bt):
                xa = xpool.tile([128, KC, BW], BF16, tag="x")
                nc.vector.tensor_tensor(
                    out=xa[:], in0=sampleT[:, :, bt * BW:(bt + 1) * BW],
                    in1=rt[:], op=AOP.is_gt)
                return xa

            # prologue: rand for t=0,1 and compares for t=0
            rts = {}
            xts = {}
            for t0 in range(min(2, T)):
                for bt in range(NBT):
                    rts[(t0, bt)] = emit_dma(t0, bt)
            for bt in range(NBT):
                xts[(0, bt)] = emit_compare(rts.pop((0, bt)), bt)

            s1t = {}       # (bt, h) -> s1_inv tile of current t
            pend_l2 = []   # deferred layer-2 emissions: [t, bt, ps2|None]

            def emit_l2(t, bt, ps2=None):
                bsl = slice(bt * BW, (bt + 1) * BW)
                ps2 = ps2pool.tile([O, BW], F32, tag="ps2")
                nc.vector.tensor_scalar(
                    ps2[:], d2[:, bsl], DECAY, rs2[:], AOP.mult, AOP.add)
                for h in range(NH):
                    nc.tensor.matmul(
                        ps2[:], w2[:, h, :], s1t[(bt, h)][:],
                        start=False, stop=(h == NH - 1),
                        skip_group_check=True)
                s2 = s2pool.tile([O, BW], F32, tag="s2")
                nc.vector.tensor_scalar(
                    s2[:], ps2[:], THRESH, None, AOP.is_le)
                nc.vector.tensor_tensor(
                    out=d2[:, bsl], in0=ps2[:], in1=s2[:], op=AOP.mult)
                nc.vector.tensor_tensor(
                    out=ssinv[:, bsl], in0=ssinv[:, bsl], in1=s2[:],
                    op=AOP.add)

            for t in range(T):
                for bt in range(NBT):
                    bsl = slice(bt * BW, (bt + 1) * BW)
                    xt = xts[(t, bt)]
                    for h in range(NH):
                        # interleave slots keep DMA/DVE fed without
                        # stalling the PE stream
                        if bt == 0 and h == 2 and t + 2 < T:
                            for b2 in range(NBT):
                                rts[(t + 2, b2)] = emit_dma(t + 2, b2)
                        if h == 4 and t + 1 < T:
                            xts[(t + 1, bt)] = emit_compare(
                                rts.pop((t + 1, bt)), bt)
                        if h == 1:
                            while pend_l2:
                                emit_l2(*pend_l2.pop(0))

                        ps = ps1pool.tile([128, BW], F32, tag="ps1")
                        nc.scalar.activation(
                            ps[:], dstate[h][:, bsl], AFT.Copy, scale=DECAY)
                        for c in range(KC):
                            nc.tensor.matmul(
                                ps[:], w1hi[:, c, h * 128:(h + 1) * 128],
                                xt[:, c, :], start=False, stop=False,
                                skip_group_check=True)
                        for c in range(KC):
                            nc.tensor.matmul(
                                ps[:], w1lo[:, c, h * 128:(h + 1) * 128],
                                xt[:, c, :], start=False,
                                stop=(c == KC - 1), skip_group_check=True)
                        s1 = s1pool.tile([128, BW], F32R, tag="s1")
                        nc.vector.tensor_scalar(
                            s1[:], ps[:], THRESH, None, AOP.is_le)
                        nc.vector.tensor_tensor(
                            out=dstate[h][:, bsl], in0=ps[:], in1=s1[:],
                            op=AOP.mult)
                        s1t[(bt, h)] = s1
                    pend_l2.append([t, bt, None])
            while pend_l2:
                emit_l2(*pend_l2.pop(0))

            # out = 1 - ssinv/T  (= sumspike/T)
            o = spool.tile([O, BL], F32, tag="o")
            nc.vector.tensor_scalar(
                o[:], ssinv[:], -1.0 / T, 1.0, AOP.mult, AOP.add)
            nc.sync.dma_start(out=d_out[:], in_=o[:])

    nc.compile()
    return nc


def _trunc_fp22(w):
    """Truncate f32 mantissa to 11 explicit bits (PE FP22 weight load)."""
    u = w.astype(np.float32).view(np.uint32)
    return (u & ~np.uint32((1 << 12) - 1)).view(np.float32)


def _prep_weights(W1, W2):
    # W1T chunks: arr[p, c, m] = W1[m, c*128+p], K zero-padded 784->896
    w1p = np.zeros((H, KPAD), np.float32)
    w1p[:, :K] = W1
    w1t = np.ascontiguousarray(
        w1p.T.reshape(KC, 128, H).transpose(1, 0, 2))        # [128, KC, H]
    if HI_MODE == "fp22":
        w1hi = _trunc_fp22(w1t)
    else:
        w1hi = w1t.astype(ml_dtypes.bfloat16).astype(np.float32)
    w1lo = (w1t.astype(np.float64) - w1hi.astype(np.float64)).astype(
        np.float32).astype(ml_dtypes.bfloat16)
    if HI_MODE != "fp22":
        w1hi = w1hi.astype(ml_dtypes.bfloat16)

    w2n = _trunc_fp22(-W2.astype(np.float32))                # fp22(-W2)
    w2t = np.ascontiguousarray(
        w2n.T.reshape(NH, 128, O).transpose(1, 0, 2))        # [128, NH, O]
    # rs2 = rowsum of effective weights W2eff = -w2n
    rs2 = (-w2n).astype(np.float64).sum(axis=1).astype(np.float32).reshape(O, 1)
    return w1hi, w1lo, w2t, rs2


def kernel(input, rand_u, W1, W2, time_window=None, _trace=False):
    input = np.asarray(input)
    rand_u = np.asarray(rand_u)
    W1 = np.asarray(W1, np.float32)
    W2 = np.asarray(W2, np.float32)
    T = rand_u.shape[0]
    assert input.shape[0] == B_FULL and rand_u.shape[2] == K

    if T not in _cache:
        _cache[T] = _build(T)
    nc = _cache[T]

    w1hi, w1lo, w2t, rs2 = _prep_weights(W1, W2)
    sample = input.reshape(B_FULL, K).astype(np.float32)

    in_maps = []
    for c in range(NCORES):
        sl = slice(c * BL, (c + 1) * BL)
        # sampleT [128, KC, BL]: sampleT[p, c, b] = sample[b, c*128+p]
        sp = np.full((BL, KPAD), -1.0, np.float32)
        sp[:, :K] = sample[sl]
        spT = np.ascontiguousarray(
            sp.T.reshape(KC, 128, BL).transpose(1, 0, 2))
        # rand [T, NBT, KPAD, BW]: pad rows = +2.0 -> x = 0 exactly
        r = rand_u[:, sl, :]                                  # [T, BL, 784]
        rp = np.full((T, KPAD, BL), 2.0, np.float32)
        rp[:, :K, :] = r.transpose(0, 2, 1)
        rbt = np.ascontiguousarray(
            rp.reshape(T, KPAD, NBT, BW).transpose(0, 2, 1, 3))
        m = {
            "rand": rbt,
            "sampleT": spT,
            "w1hi": w1hi,
            "w1lo": w1lo,
            "w2": w2t,
            "rs2": rs2,
        }
        in_maps.append(m)

    res = run_bass_kernel_spmd(nc, in_maps, list(range(NCORES)), trace=_trace)
    out = np.empty((B_FULL, O), np.float32)
    for c in range(NCORES):
        out[c * BL:(c + 1) * BL, :] = res.results[c]["out"].T
    if _trace:
        return out, res
    return out


# revision 4
# speedup vs baseline: 1.0006x; 1.0006x over previous
"""Trainium2 Bass kernel for the Event-SMLP-Quantized SNN — v2.

Data-parallel over batch: 1024 per core x 8 cores. Changes vs v1:
  - rand_u transposed to feature-major on the HOST -> no on-device
    transposes; the is_gt compare directly yields xT [k, b] tiles.
  - Layer-2 runs a single fp32r pass (PE truncates f32->FP22; W2 is
    pre-truncated so products are exact w.r.t. the truncated weights).
  - Layer-1 hi split uses fp22-truncated W1 via fp32r (exact products),
    lo split = bf16 residual -> product error ~2^-21 (vs 2^-17 for the
    plain 2-way bf16 split).
  - Software-pipelined emission: next-t compares and DMAs are slotted
    mid-timestep to keep the PE stream gapless.

Per timestep, per core:
  x = (sample > rand_t)            DVE compare, feature-major, bf16 out
  u = 0.2*m1 + x @ W1.T            ACT preload + 14 PE matmuls / h-tile
  s1_inv = (u <= 0.5); m1 = u*s1_inv
  z2 = rowsum(W2eff) + 0.2*m2 + s1_inv @ (-W2eff).T   (8 fp32r matmuls)
  s2_inv = (z2 <= 0.5); m2 = z2*s2_inv; ssinv += s2_inv
  out = 1 - ssinv/T
"""
import os
import sys

sys.path.insert(0, "/opt/trn_rl_repo")

import numpy as np
import ml_dtypes

import concourse.bacc as bacc
import concourse.mybir as mybir
from concourse import tile
from concourse.bass_utils import run_bass_kernel_spmd

F32 = mybir.dt.float32
F32R = mybir.dt.float32r
BF16 = mybir.dt.bfloat16
AOP = mybir.AluOpType
AFT = mybir.ActivationFunctionType

NCORES = 8
B_FULL = 8192
BL = B_FULL // NCORES        # 1024 batch per core
NBT = 2                      # matmul N-tiles of 512
BW = 512
K = 784
KPAD = 896                   # 7 * 128
KC = KPAD // 128             # 7 contraction chunks
H = 1024
NH = H // 128                # 8 hidden tiles
O = 10
DECAY = 0.2
THRESH = 0.5

# L1 hi-split carried as fp32r (fp22-exact) when trunc model confirmed,
# else plain bf16. Set by experiment results.
HI_MODE = os.environ.get("KERNEL_HI_MODE", "bf16")   # "fp22" | "bf16"

_cache = {}


def _build(T):
    nc = bacc.Bacc("TRN2", target_bir_lowering=False, debug=False)

    d_rand = nc.declare_dram_parameter("rand", [T, NBT, KPAD, BW], F32, isOutput=False)
    d_sample = nc.declare_dram_parameter("sampleT", [128, KC, BL], F32, isOutput=False)
    hi_dt = F32R if HI_MODE == "fp22" else BF16
    d_w1hi = nc.declare_dram_parameter("w1hi", [128, KC, H], hi_dt, isOutput=False)
    d_w1lo = nc.declare_dram_parameter("w1lo", [128, KC, H], BF16, isOutput=False)
    d_w2 = nc.declare_dram_parameter("w2", [128, NH, O], F32R, isOutput=False)
    d_rs2 = nc.declare_dram_parameter("rs2", [O, 1], F32, isOutput=False)
    d_out = nc.declare_dram_parameter("out", [O, BL], F32, isOutput=True)

    with tile.TileContext(nc) as tc:
        with (
            tc.tile_pool(name="const", bufs=1) as cpool,
            tc.tile_pool(name="state", bufs=1) as spool,
            tc.tile_pool(name="rand", bufs=3) as rpool,
            tc.tile_pool(name="x", bufs=3) as xpool,
            tc.tile_pool(name="s1", bufs=10) as s1pool,
            tc.tile_pool(name="s2", bufs=4) as s2pool,
            tc.tile_pool(name="dec", bufs=2) as decpool,
            tc.tile_pool(name="ps1", bufs=6, space="PSUM") as ps1pool,
            tc.tile_pool(name="ps2", bufs=2, space="PSUM") as ps2pool,
        ):
            sampleT = cpool.tile([128, KC, BL], F32, tag="sampleT")
            nc.sync.dma_start(out=sampleT[:], in_=d_sample[:])
            w1hi = cpool.tile([128, KC, H], hi_dt, tag="w1hi")
            nc.sync.dma_start(out=w1hi[:], in_=d_w1hi[:])
            w1lo = cpool.tile([128, KC, H], BF16, tag="w1lo")
            nc.sync.dma_start(out=w1lo[:], in_=d_w1lo[:])
            w2 = cpool.tile([128, NH, O], F32R, tag="w2")
            nc.sync.dma_start(out=w2[:], in_=d_w2[:])
            rs2 = cpool.tile([O, 1], F32, tag="rs2")
            nc.sync.dma_start(out=rs2[:], in_=d_rs2[:])

            dstate = []
            for h in range(NH):
                d_ = spool.tile([128, BL], F32, tag=f"dm{h}")
                nc.vector.memset(d_[:], 0.0)
                dstate.append(d_)
            d2 = spool.tile([O, BL], F32, tag="dl2")
            nc.vector.memset(d2[:], 0.0)
            ssinv = spool.tile([O, BL], F32, tag="ssinv")
            nc.vector.memset(ssinv[:], 0.0)

            def emit_dma(t, bt):
                rt = rpool.tile([128, KC, BW], F32, tag="rand")
                src = d_rand[t, bt].rearrange("(c p) b -> p c b", p=128)
                nc.sync.dma_start(out=rt[:], in_=src)
                return rt

            def emit_compare(rt, bt):
                xa = xpool.tile([128, KC, BW], BF16, tag="x")
                nc.vector.tensor_tensor(
                    out=xa[:], in0=sampleT[:, :, bt * BW:(bt + 1) * BW],
                    in1=rt[:], op=AOP.is_gt)
                return xa

            # prologue: rand for t=0,1 and compares for t=0
            rts = {}
            xts = {}
            for t0 in range(min(2, T)):
                for bt in range(NBT):
                    rts[(t0, bt)] = emit_dma(t0, bt)
            for bt in range(NBT):
                xts[(0, bt)] = emit_compare(rts.pop((0, bt)), bt)

            s1t = {}       # (bt, h) -> s1_inv tile of current t
            pend_l2 = []   # deferred layer-2 emissions: [t, bt, ps2|None]

            def emit_l2(t, bt, ps2=None):
                # matmuls start from zero PSUM -> the PE never waits on a
                # DVE preload; decay+rowsum joins on the SBUF side below
                bsl = slice(bt * BW, (bt + 1) * BW)
                dec2 = decpool.tile([O, BW], F32, tag="dec2")
                nc.vector.tensor_scalar(
                    dec2[:], d2[:, bsl], DECAY, rs2[:], AOP.mult, AOP.add)
                ps2 = ps2pool.tile([O, BW], F32, tag="ps2")
                for h in range(NH):
                    nc.tensor.matmul(
                        ps2[:], w2[:, h, :], s1t[(bt, h)][:],
                        start=(h == 0), stop=(h == NH - 1),
                        skip_group_check=True)
                nc.vector.tensor_tensor(
                    out=dec2[:], in0=ps2[:], in1=dec2[:], op=AOP.add)
                s2 = s2pool.tile([O, BW], F32, tag="s2")
                nc.vector.tensor_scalar(
                    s2[:], dec2[:], THRESH, None, AOP.is_le)
                nc.vector.tensor_tensor(
                    out=d2[:, bsl], in0=dec2[:], in1=s2[:], op=AOP.mult)
                nc.vector.tensor_tensor(
                    out=ssinv[:, bsl], in0=ssinv[:, bsl], in1=s2[:],
                    op=AOP.add)

            for t in range(T):
                for bt in range(NBT):
                    bsl = slice(bt * BW, (bt + 1) * BW)
                    xt = xts[(t, bt)]
                    for h in range(NH):
                        # interleave slots keep DMA/DVE fed without
                        # stalling the PE stream
                        if bt == 0 and h == 2 and t + 2 < T:
                            for b2 in range(NBT):
                                rts[(t + 2, b2)] = emit_dma(t + 2, b2)
                        if h == 4 and t + 1 < T:
                            xts[(t + 1, bt)] = emit_compare(
                                rts.pop((t + 1, bt)), bt)
                        if h == 1:
                            while pend_l2:
                                emit_l2(*pend_l2.pop(0))

                        ps = ps1pool.tile([128, BW], F32, tag="ps1")
                        nc.scalar.activation(
                            ps[:], dstate[h][:, bsl], AFT.Copy, scale=DECAY)
                        for c in range(KC):
                            nc.tensor.matmul(
                                ps[:], w1hi[:, c, h * 128:(h + 1) * 128],
                                xt[:, c, :], start=False, stop=False,
                                skip_group_check=True)
                        for c in range(KC):
                            nc.tensor.matmul(
                                ps[:], w1lo[:, c, h * 128:(h + 1) * 128],
                                xt[:, c, :], start=False,
                                stop=(c == KC - 1), skip_group_check=True)
                        s1 = s1pool.tile([128, BW], F32R, tag="s1")
                        nc.vector.tensor_scalar(
                            s1[:], ps[:], THRESH, None, AOP.is_le)
                        nc.vector.tensor_tensor(
                            out=dstate[h][:, bsl], in0=ps[:], in1=s1[:],
                            op=AOP.mult)
                        s1t[(bt, h)] = s1
                    pend_l2.append([t, bt, None])
            while pend_l2:
                emit_l2(*pend_l2.pop(0))

            # out = 1 - ssinv/T  (= sumspike/T)
            o = spool.tile([O, BL], F32, tag="o")
            nc.vector.tensor_scalar(
                o[:], ssinv[:], -1.0 / T, 1.0, AOP.mult, AOP.add)
            nc.sync.dma_start(out=d_out[:], in_=o[:])

    nc.compile()
    return nc


def _trunc_fp22(w):
    """Truncate f32 mantissa to 11 explicit bits (PE FP22 weight load)."""
    u = w.astype(np.float32).view(np.uint32)
    return (u & ~np.uint32((1 << 12) - 1)).view(np.float32)


def _prep_weights(W1, W2):
    # W1T chunks: arr[p, c, m] = W1[m, c*128+p], K zero-padded 784->896
    w1p = np.zeros((H, KPAD), np.float32)
    w1p[:, :K] = W1
    w1t = np.ascontiguousarray(
        w1p.T.reshape(KC, 128, H).transpose(1, 0, 2))        # [128, KC, H]
    if HI_MODE == "fp22":
        w1hi = _trunc_fp22(w1t)
    else:
        w1hi = w1t.astype(ml_dtypes.bfloat16).astype(np.float32)
    w1lo = (w1t.astype(np.float64) - w1hi.astype(np.float64)).astype(
        np.float32).astype(ml_dtypes.bfloat16)
    if HI_MODE != "fp22":
        w1hi = w1hi.astype(ml_dtypes.bfloat16)

    w2n = _trunc_fp22(-W2.astype(np.float32))                # fp22(-W2)
    w2t = np.ascontiguousarray(
        w2n.T.reshape(NH, 128, O).transpose(1, 0, 2))        # [128, NH, O]
    # rs2 = rowsum of effective weights W2eff = -w2n
    rs2 = (-w2n).astype(np.float64).sum(axis=1).astype(np.float32).reshape(O, 1)
    return w1hi, w1lo, w2t, rs2


def kernel(input, rand_u, W1, W2, time_window=None, _trace=False):
    input = np.asarray(input)
    rand_u = np.asarray(rand_u)
    W1 = np.asarray(W1, np.float32)
    W2 = np.asarray(W2, np.float32)
    T = rand_u.shape[0]
    assert input.shape[0] == B_FULL and rand_u.shape[2] == K

    if T not in _cache:
        _cache[T] = _build(T)
    nc = _cache[T]

    w1hi, w1lo, w2t, rs2 = _prep_weights(W1, W2)
    sample = input.reshape(B_FULL, K).astype(np.float32)

    in_maps = []
    for c in range(NCORES):
        sl = slice(c * BL, (c + 1) * BL)
        # sampleT [128, KC, BL]: sampleT[p, c, b] = sample[b, c*128+p]
        sp = np.full((BL, KPAD), -1.0, np.float32)
        sp[:, :K] = sample[sl]
        spT = np.ascontiguousarray(
            sp.T.reshape(KC, 128, BL).transpose(1, 0, 2))
        # rand [T, NBT, KPAD, BW]: pad rows = +2.0 -> x = 0 exactly
        r = rand_u[:, sl, :]                                  # [T, BL, 784]
        rp = np.full((T, KPAD, BL), 2.0, np.float32)
        rp[:, :K, :] = r.transpose(0, 2, 1)
        rbt = np.ascontiguousarray(
            rp.reshape(T, KPAD, NBT, BW).transpose(0, 2, 1, 3))
        m = {
            "rand": rbt,
            "sampleT": spT,
            "w1hi": w1hi,
            "w1lo": w1lo,
            "w2": w2t,
            "rs2": rs2,
        }
        in_maps.append(m)

    res = run_bass_kernel_spmd(nc, in_maps, list(range(NCORES)), trace=_trace)
    out = np.empty((B_FULL, O), np.float32)
    for c in range(NCORES):
        out[c * BL:(c + 1) * BL, :] = res.results[c]["out"].T
    if _trace:
        return out, res
    return out
